# revision 32
# baseline (speedup 1.0000x reference)
"""Trainium2 Bass kernel for sorted segment_max (ClusterPool).

Problem: features [2M, 128] f32, segment_ids [2M] sorted int, num_clusters
10000 -> output [10000, 128] = per-cluster max over rows.

Strategy (8 NeuronCores, SPMD):
  - Shard rows: core c gets rows [c*250k, (c+1)*250k).  Sorted ids => each
    core covers a contiguous cluster range (~1252 clusters), padded to 1280
    local clusters = 10 batches x 128.
  - MAIN pass: cluster rows covered by the 8-row-aligned blocks fully inside
    [s, e).  Clusters sorted by aligned-block count nb so every batch of 128
    has near-uniform nb (padding ~1.03x).  Batch 0 (smallest nb, incl. all
    nb<4) gathers 8-row 4KB elements (elem_step=1024 = block index).  Batches
    1+ (all nb>=4) gather 32-row 16KB elements with elem_step=1024: anchor
    t for cluster p is a0 + min(4t, nb-4), i.e. 4-block strides with the
    last element tail-anchored (overlap re-reads are harmless for max).
    This cuts SWDGE descriptor count ~4x, which matters because descriptor
    ring writes share an SBUF port with 2-port DVE ops.
  - Per chunk of <=2 (32-row) or <=8 (8-row) elements: 2-port tensor_tensor
    max halvings down to 8 rows (1.05 ns/elem), one strided tensor_reduce
    8->1, then a [128,128] accumulate max.
  - BOUNDARY pass: clusters with cnt>=8 are finished by two row-granular
    4KB elements (elem_step=128): head rows [s, s+8) and tail rows [e-8, e)
    (the three ranges overlap; duplicates are fine for max).  Consecutive
    cluster batches fit the 32768-row int16 window.  Clusters with cnt<8
    are computed on the host (only shard-edge clusters qualify).
  - Host combines main partials (sorted order), boundary partials, host
    tiny-cluster rows across the 8 cores with np.maximum; empty -> -inf.
"""

import os
import sys

import numpy as np

sys.path.insert(0, "/opt/trn_rl_repo")

N_POINTS = 2_000_000
D = 128
N_CLUSTERS = 10_000
N_CORES = 8
RPC = N_POINTS // N_CORES  # rows per core
NBLKS = RPC // 8  # 8-row blocks per core (31250 < 2^15)
NCL = 1280  # padded local clusters per core
NBATCH = NCL // 128
WINDOW = 32768  # row window for the boundary gather
NB2 = 2  # boundary elements per cluster (head + tail)

_last_results = None  # BassKernelResults of the most recent run (for test.py)


def _apply_drain_patch():
    """walrus TPB_CTRL supports a single sync wait; TileContext's tail drain
    accumulates one wait per outstanding proc.  Split them across NOPs."""
    import concourse.mybir as mybir
    import concourse.tile as tile
    from concourse.vector_clock import ScopedClock

    if getattr(tile.TileContext, "_drain_patched", False):
        return

    def _patched(self, tick_clock, wait_clock):
        nc = self.nc
        nop = nc.sync.nop(nofuse=True, hint="tail_drain_waits")
        wait_clock.add_sem_waits(nop.ins, ScopedClock({None: tick_clock.global_clock}))
        si = nop.ins.sync_info
        waits = list(si.on_wait) if si is not None and si.on_wait else []
        if len(waits) > 1:
            si.on_wait = waits[:1]
            for i in range(1, len(waits)):
                extra = nc.sync.nop(nofuse=True, hint=f"tail_drain_waits_{i}")
                if extra.ins.sync_info is None:
                    extra.ins.sync_info = mybir.SyncInfo(
                        on_wait=waits[i : i + 1], on_update=[]
                    )
                else:
                    extra.ins.sync_info.on_wait = waits[i : i + 1]
        nc.sync.drain()
        nc.all_engine_barrier()
        assert self.sems is not None
        popped = nc._tile_sem_poison_stack.pop()
        assert popped is self._sem_poison
        nc.clear_and_free_semaphores(list(self.sems.allocated().values()))
        nc.all_engine_barrier()

    tile.TileContext._drain_and_barrier = _patched
    tile.TileContext._drain_patched = True


def _build_program(tms, bwindows):
    """Build the SPMD Bass program.

    tms[m]      = (rows_per_elem, T) for sorted batch m: T gather elements
                  per cluster, each covering rows_per_elem rows
    bwindows[m] = boundary-pass window base row for consecutive batch m"""
    import concourse.bacc as bacc
    import concourse.mybir as mybir
    import concourse.tile as tile
    from concourse.bass import AP

    _apply_drain_patch()

    CM = max(sum(t for _, t in ps) for ps in tms) * 8  # idx cols per batch

    nc = bacc.Bacc(None, num_swdge_queues=4)
    f_in = nc.dram_tensor("features", [RPC, D], mybir.dt.float32, kind="ExternalInput")
    mi_in = nc.dram_tensor(
        "midx", [NBATCH, 128, CM], mybir.dt.int16, kind="ExternalInput"
    )
    bi_in = nc.dram_tensor(
        "bidx", [NBATCH, 128, NB2 * 8], mybir.dt.int16, kind="ExternalInput"
    )
    pm_out = nc.dram_tensor(
        "pmain", [NCL, D], mybir.dt.float32, kind="ExternalOutput"
    )
    pb_out = nc.dram_tensor(
        "pbnd", [NCL, D], mybir.dt.float32, kind="ExternalOutput"
    )

    fbase = f_in[:, :]
    # 8-row-block-index source views (ap[0][0] = elem_step, ap[-1][1] = elem)
    blocks8 = AP(fbase.tensor, 0, [[1024, NBLKS], [1, 1024]])
    blocks32 = AP(fbase.tensor, 0, [[1024, NBLKS - 3], [1, 4096]])
    gq = 0
    with tile.TileContext(nc) as tc:
        with (
            tc.tile_pool(name="gp", bufs=5) as gp,
            tc.tile_pool(name="bp", bufs=2) as bp,
            tc.tile_pool(name="ip", bufs=12) as ip,
            tc.tile_pool(name="sp", bufs=2) as sp,
        ):
            def boundary_gather(m):
                # ---- boundary pass: consecutive batch m, head+tail 8 rows.
                # The gather is issued at the START of the batch so the q3
                # transfer has a whole batch of slack before its DVE ops run
                # (late boundary data would head-of-line block the DVE).
                w = bwindows[m]
                win = AP(fbase.tensor, w * D, [[D, WINDOW - 7], [1, 1024]])
                bt = ip.tile([128, NB2 * 8], mybir.dt.int16, tag="bidx")
                nc.sync.dma_start(out=bt[:], in_=bi_in[m])
                bg = bp.tile([128, NB2 * 1024], mybir.dt.float32, tag="bg")
                nonlocal gq
                nc.gpsimd.dma_gather(
                    out_ap=bg[:].rearrange("p (t e) -> p t e", e=1024),
                    in_ap=win,
                    idxs_ap=bt[:],
                    num_idxs=NB2 * 128,
                    num_idxs_reg=NB2 * 128,
                    elem_size=1024,
                    elem_step=128,
                    queue_num=3,
                    single_packet=False,
                )
                gq += 1
                return bg

            def boundary_reduce(m, bg):
                br = sp.tile([128, D], mybir.dt.float32, tag="bred")
                nc.vector.tensor_tensor(
                    out=bg[:, : 8 * D], in0=bg[:, : 8 * D],
                    in1=bg[:, 8 * D : 16 * D],
                    op=mybir.AluOpType.max,
                )
                nc.vector.tensor_reduce(
                    out=br[:],
                    in_=bg[:, : 8 * D].rearrange("p (t d) -> p d t", d=D),
                    axis=mybir.AxisListType.X,
                    op=mybir.AluOpType.max,
                )
                nc.sync.dma_start(out=pb_out[m * 128 : (m + 1) * 128, :], in_=br[:])

            for m in range(NBATCH):
                # ---- main pass: sorted batch m, passes [(rpe, T), ...] ---
                it = ip.tile([128, CM], mybir.dt.int16, tag="midx")
                nc.sync.dma_start(out=it[:], in_=mi_in[m])
                bg = boundary_gather(m)
                acc = None
                coff = 0  # column offset into the index table
                ri = 0
                for rpe, T in tms[m]:
                    if T == 0:
                        continue
                    elem = rpe * D  # floats per gather element
                    thmax = 8192 // elem  # elements per chunk (32KB tile)
                    src = blocks8 if rpe == 8 else blocks32
                    chunks = [
                        (c0, min(thmax, T - c0)) for c0 in range(0, T, thmax)
                    ]
                    # split the first batch's first call into a small primer
                    # so the first reduce starts as early as possible
                    if m == 0 and acc is None and chunks[0][1] > 1:
                        chunks = [(0, 1), (1, chunks[0][1] - 1)] + chunks[1:]
                    for c0, TH in chunks:
                        g = gp.tile([128, 8192], mybir.dt.float32, tag="gath")
                        nc.gpsimd.dma_gather(
                            out_ap=g[:, : TH * elem].rearrange(
                                "p (t e) -> p t e", e=elem
                            ),
                            in_ap=src,
                            idxs_ap=it[:, coff + c0 * 8 : coff + (c0 + TH) * 8],
                            num_idxs=TH * 128,
                            num_idxs_reg=TH * 128,
                            elem_size=elem,
                            elem_step=1024,
                            # first calls stay on one queue: FIFO completion,
                            # so the pipeline ramps instead of all transfers
                            # sharing bandwidth and finishing together
                            queue_num=0 if gq < 5 else gq % 3,
                            single_packet=False,
                        )
                        gq += 1
                        r = sp.tile([128, D], mybir.dt.float32, tag=f"red{ri % 8}")
                        ri += 1
                        rows = TH * rpe
                        # 2-port TT halvings (1.05 ns/elem) instead of strided
                        # tensor_reduce (1.67 ns/elem)
                        while rows % 2 == 0 and rows > 8:
                            half = rows // 2 * D
                            nc.vector.tensor_tensor(
                                out=g[:, :half], in0=g[:, :half],
                                in1=g[:, half : 2 * half],
                                op=mybir.AluOpType.max,
                            )
                            rows //= 2
                        nc.vector.tensor_reduce(
                            out=r[:],
                            in_=g[:, : rows * D].rearrange(
                                "p (t d) -> p d t", d=D
                            ),
                            axis=mybir.AxisListType.X,
                            op=mybir.AluOpType.max,
                        )
                        if acc is None:
                            acc = r
                        else:
                            nc.vector.tensor_tensor(
                                out=acc[:], in0=acc[:], in1=r[:],
                                op=mybir.AluOpType.max,
                            )
                    coff += T * 8
                nc.sync.dma_start(out=pm_out[m * 128 : (m + 1) * 128, :], in_=acc[:])
                boundary_reduce(m, bg)

    if not nc.is_finalized():
        nc.finalize()
    return nc


def kernel(features, segment_ids, num_clusters):
    global _last_results
    from concourse.bass_utils import run_bass_kernel_spmd

    features = np.ascontiguousarray(np.asarray(features, dtype=np.float32))
    ids = np.asarray(segment_ids).astype(np.int64)
    nclusters = int(num_clusters)
    assert features.shape == (N_POINTS, D), features.shape
    assert ids.shape == (N_POINTS,)
    assert nclusters == N_CLUSTERS

    # --- host cluster metadata -------------------------------------------
    gstart = np.searchsorted(ids, np.arange(nclusters), side="left")
    gend = np.searchsorted(ids, np.arange(nclusters) + 1, side="left")
    gcounts = gend - gstart

    core_meta = []  # per core dict
    for c in range(N_CORES):
        r0, r1 = c * RPC, (c + 1) * RPC
        cl_lo, cl_hi = int(ids[r0]), int(ids[r1 - 1])
        ncl = cl_hi - cl_lo + 1
        assert ncl <= NCL, f"core {c}: {ncl} local clusters > {NCL}"
        s = np.clip(gstart[cl_lo : cl_hi + 1], r0, r1) - r0
        e = np.clip(gend[cl_lo : cl_hi + 1], r0, r1) - r0
        s_pad = np.zeros(NCL, dtype=np.int64)
        e_pad = np.zeros(NCL, dtype=np.int64)
        s_pad[:ncl] = s
        e_pad[:ncl] = e
        cnt = e_pad - s_pad
        a0 = (s_pad + 7) // 8  # first aligned block fully inside
        a1 = e_pad // 8  # one past last aligned block fully inside
        nb = np.maximum(a1 - a0, 0)
        nb[cnt == 0] = 0
        order = np.argsort(nb, kind="stable")  # sorted batches for main pass
        core_meta.append(
            dict(cl_lo=cl_lo, ncl=ncl, s=s_pad, e=e_pad, cnt=cnt,
                 a0=a0, nb=nb, order=order)
        )

    # Main-pass per-batch passes shared across cores.  Batch 0 holds every
    # nb<4 cluster -> 8-row elements; batches 1+ are all nb>=4 -> 32-row
    # elements with T = max ceil(nb/4), the last element tail-anchored.
    tms = []
    for m in range(NBATCH):
        if m == 0:
            T = 1
            for cm in core_meta:
                sel = cm["order"][:128]
                T = max(T, int(cm["nb"][sel].max()))
            tms.append([(8, T)])
        else:
            T = 1
            for cm in core_meta:
                sel = cm["order"][m * 128 : (m + 1) * 128]
                nb = cm["nb"][sel]
                assert int(nb.min()) >= 4, (m, int(nb.min()))
                T = max(T, int(-(-nb.max() // 4)))
            tms.append([(32, T)])
    CM = max(sum(t for _, t in ps) for ps in tms) * 8

    # Boundary windows (consecutive batches, shared across cores).
    bwindows = []
    for m in range(NBATCH):
        jj = slice(m * 128, (m + 1) * 128)
        wmin = RPC
        for cm in core_meta:
            act = cm["cnt"][jj] >= 8
            if act.any():
                wmin = min(wmin, int(cm["s"][jj][act].min()))
        w = max(0, min(wmin, RPC - WINDOW))
        bwindows.append(w)

    def wrap_calls(V, T):
        """V[j] with j = t*128+p for t in [0,T) -> wrapped int16 [128, T*8]."""
        tab = V.reshape(T * 8, 16).T  # [16, T*8]; col m covers V[m*16:(m+1)*16]
        return np.tile(tab, (8, 1))

    # --- main-pass tables -------------------------------------------------
    midx_all, bidx_all = [], []
    for cm in core_meta:
        g = np.zeros((NBATCH, 128, CM), dtype=np.int16)
        for m in range(NBATCH):
            sel = cm["order"][m * 128 : (m + 1) * 128]
            a0 = cm["a0"][sel]
            nb = cm["nb"][sel]
            coff = 0
            for rpe, T in tms[m]:
                if T == 0:
                    continue
                t = np.arange(T)[None, :]
                if rpe == 32:
                    # anchor t: a0 + min(4t, nb-4); covers [a0, a1) exactly
                    # with the final element tail-anchored, pads repeat last
                    blk = a0[:, None] + np.minimum(4 * t, (nb - 4)[:, None])
                    assert blk.max() < NBLKS - 3
                else:
                    blk = a0[:, None] + t
                    last = np.where(nb > 0, a0 + nb - 1, 0)
                    blk = np.where(t >= nb[:, None], last[:, None], blk)
                    blk = np.where((nb[:, None] > 0), blk, 0)
                    assert blk.max() < NBLKS
                assert blk.min() >= 0
                V = blk.astype(np.int16).T.reshape(-1)  # j = t*128 + p
                g[m, :, coff : coff + T * 8] = wrap_calls(V, T)
                coff += T * 8
        midx_all.append(g)

        # boundary tables: consecutive batches, head+tail 8-row anchors
        b = np.zeros((NBATCH, 128, NB2 * 8), dtype=np.int16)
        for m in range(NBATCH):
            jj = slice(m * 128, (m + 1) * 128)
            s = cm["s"][jj]
            e = cm["e"][jj]
            cnt = cm["cnt"][jj]
            w = bwindows[m]
            ok = cnt >= 8
            head = np.where(ok, s - w, 0)
            tail = np.where(ok, e - 8 - w, 0)
            rel = np.stack([head, tail])  # [2, 128], j = t*128+p
            assert rel.min() >= 0 and rel.max() < WINDOW - 7, (m, rel.min(), rel.max())
            b[m] = wrap_calls(rel.astype(np.int16).reshape(-1), NB2)
        bidx_all.append(b)

    # --- build + run ------------------------------------------------------
    nc = _build_program(tms, bwindows)
    in_maps = [
        {
            "features": features[c * RPC : (c + 1) * RPC],
            "midx": midx_all[c],
            "bidx": bidx_all[c],
        }
        for c in range(N_CORES)
    ]
    res = run_bass_kernel_spmd(nc, in_maps, list(range(N_CORES)))
    _last_results = res

    # --- host combine -----------------------------------------------------
    full = np.full((nclusters, D), -np.inf, dtype=np.float32)
    for c in range(N_CORES):
        cm = core_meta[c]
        cl_lo, ncl = cm["cl_lo"], cm["ncl"]
        pm = res.results[c]["pmain"]  # [NCL, D] in sorted order
        pb = res.results[c]["pbnd"]  # [NCL, D] in consecutive order
        order = cm["order"]
        nb_sorted = cm["nb"][order]
        valid_m = (nb_sorted > 0) & (order < ncl)
        rows = cl_lo + order[valid_m]
        np.maximum.at(full, rows, pm[valid_m])
        valid_b = (cm["cnt"][:NCL] >= 8) & (np.arange(NCL) < ncl)
        rows = cl_lo + np.nonzero(valid_b)[0]
        np.maximum.at(full, rows, pb[valid_b])
        # tiny clusters (0 < cnt < 8, shard edges only): host-computed
        tiny = np.nonzero((cm["cnt"][:NCL] > 0) & (cm["cnt"][:NCL] < 8)
                          & (np.arange(NCL) < ncl))[0]
        r0 = c * RPC
        for j in tiny:
            rows_f = features[r0 + cm["s"][j] : r0 + cm["e"][j]]
            full[cl_lo + j] = np.maximum(full[cl_lo + j], rows_f.max(axis=0))
    full[gcounts == 0] = -np.inf
    return full


# revision 33
# speedup vs baseline: 1.1837x; 1.1837x over previous
"""Trainium2 Bass kernel for sorted segment_max (ClusterPool).

Problem: features [2M, 128] f32, segment_ids [2M] sorted int, num_clusters
10000 -> output [10000, 128] = per-cluster max over rows.

Strategy (8 NeuronCores, SPMD):
  - Shard rows: core c gets rows [c*250k, (c+1)*250k).  Sorted ids => each
    core covers a contiguous cluster range (~1252 clusters), padded to 1280
    local clusters = 10 batches x 128.
  - MAIN pass: cluster rows covered by the 8-row-aligned blocks fully inside
    [s, e).  Clusters sorted by aligned-block count nb so every batch of 128
    has near-uniform nb (padding ~1.03x).  Batch 0 (smallest nb, incl. all
    nb<4) gathers 8-row 4KB elements (elem_step=1024 = block index).  Batches
    1+ (all nb>=4) gather 32-row 16KB elements with elem_step=1024: anchor
    t for cluster p is a0 + min(4t, nb-4), i.e. 4-block strides with the
    last element tail-anchored (overlap re-reads are harmless for max).
    This cuts SWDGE descriptor count ~4x, which matters because descriptor
    ring writes share an SBUF port with 2-port DVE ops.
  - Per chunk of <=2 (32-row) or <=8 (8-row) elements: 2-port tensor_tensor
    max halvings down to 8 rows (1.05 ns/elem), one strided tensor_reduce
    8->1, then a [128,128] accumulate max.
  - BOUNDARY pass: clusters with cnt>=8 are finished by two row-granular
    4KB elements (elem_step=128): head rows [s, s+8) and tail rows [e-8, e)
    (the three ranges overlap; duplicates are fine for max).  Consecutive
    cluster batches fit the 32768-row int16 window.  Clusters with cnt<8
    are computed on the host (only shard-edge clusters qualify).
  - Host combines main partials (sorted order), boundary partials, host
    tiny-cluster rows across the 8 cores with np.maximum; empty -> -inf.
"""

import os
import sys

import numpy as np

sys.path.insert(0, "/opt/trn_rl_repo")

N_POINTS = 2_000_000
D = 128
N_CLUSTERS = 10_000
N_CORES = 8
RPC = N_POINTS // N_CORES  # rows per core
NBLKS = RPC // 8  # 8-row blocks per core (31250 < 2^15)
NCL = 1280  # padded local clusters per core
NBATCH = NCL // 128
WINDOW = 32768  # row window for the boundary gather
NB2 = 2  # boundary elements per cluster (head + tail)

_last_results = None  # BassKernelResults of the most recent run (for test.py)


def _apply_drain_patch():
    """walrus TPB_CTRL supports a single sync wait; TileContext's tail drain
    accumulates one wait per outstanding proc.  Split them across NOPs."""
    import concourse.mybir as mybir
    import concourse.tile as tile
    from concourse.vector_clock import ScopedClock

    if getattr(tile.TileContext, "_drain_patched", False):
        return

    def _patched(self, tick_clock, wait_clock):
        nc = self.nc
        nop = nc.sync.nop(nofuse=True, hint="tail_drain_waits")
        wait_clock.add_sem_waits(nop.ins, ScopedClock({None: tick_clock.global_clock}))
        si = nop.ins.sync_info
        waits = list(si.on_wait) if si is not None and si.on_wait else []
        if len(waits) > 1:
            si.on_wait = waits[:1]
            for i in range(1, len(waits)):
                extra = nc.sync.nop(nofuse=True, hint=f"tail_drain_waits_{i}")
                if extra.ins.sync_info is None:
                    extra.ins.sync_info = mybir.SyncInfo(
                        on_wait=waits[i : i + 1], on_update=[]
                    )
                else:
                    extra.ins.sync_info.on_wait = waits[i : i + 1]
        nc.sync.drain()
        nc.all_engine_barrier()
        assert self.sems is not None
        popped = nc._tile_sem_poison_stack.pop()
        assert popped is self._sem_poison
        nc.clear_and_free_semaphores(list(self.sems.allocated().values()))
        nc.all_engine_barrier()

    tile.TileContext._drain_and_barrier = _patched
    tile.TileContext._drain_patched = True


def _build_program(tms, bwindows):
    """Build the SPMD Bass program.

    tms[m]      = (rows_per_elem, T) for sorted batch m: T gather elements
                  per cluster, each covering rows_per_elem rows
    bwindows[m] = boundary-pass window base row for consecutive batch m"""
    import concourse.bacc as bacc
    import concourse.mybir as mybir
    import concourse.tile as tile
    from concourse.bass import AP

    _apply_drain_patch()

    CM = max(sum(t for _, t in ps) for ps in tms) * 8  # idx cols per batch

    nc = bacc.Bacc(None, num_swdge_queues=4)
    f_in = nc.dram_tensor("features", [RPC, D], mybir.dt.float32, kind="ExternalInput")
    mi_in = nc.dram_tensor(
        "midx", [NBATCH, 128, CM], mybir.dt.int16, kind="ExternalInput"
    )
    bi_in = nc.dram_tensor(
        "bidx", [NBATCH, 128, NB2 * 8], mybir.dt.int16, kind="ExternalInput"
    )
    pm_out = nc.dram_tensor(
        "pmain", [NCL, D], mybir.dt.float32, kind="ExternalOutput"
    )
    pb_out = nc.dram_tensor(
        "pbnd", [NCL, D], mybir.dt.float32, kind="ExternalOutput"
    )

    fbase = f_in[:, :]
    # 8-row-block-index source views (ap[0][0] = elem_step, ap[-1][1] = elem)
    blocks8 = AP(fbase.tensor, 0, [[1024, NBLKS], [1, 1024]])
    blocks32 = AP(fbase.tensor, 0, [[1024, NBLKS - 3], [1, 4096]])
    gq = 0
    with tile.TileContext(nc) as tc:
        with (
            tc.tile_pool(name="gp", bufs=5) as gp,
            tc.tile_pool(name="bp", bufs=2) as bp,
            tc.tile_pool(name="ip", bufs=12) as ip,
            tc.tile_pool(name="sp", bufs=2) as sp,
        ):
            def boundary(m):
                # ---- boundary pass: consecutive batch m, head+tail 8 rows -
                w = bwindows[m]
                win = AP(fbase.tensor, w * D, [[D, WINDOW - 7], [1, 1024]])
                bt = ip.tile([128, NB2 * 8], mybir.dt.int16, tag="bidx")
                nc.sync.dma_start(out=bt[:], in_=bi_in[m])
                bg = bp.tile([128, NB2 * 1024], mybir.dt.float32, tag="bg")
                nonlocal gq
                nc.gpsimd.dma_gather(
                    out_ap=bg[:].rearrange("p (t e) -> p t e", e=1024),
                    in_ap=win,
                    idxs_ap=bt[:],
                    num_idxs=NB2 * 128,
                    num_idxs_reg=NB2 * 128,
                    elem_size=1024,
                    elem_step=128,
                    queue_num=3,
                    single_packet=False,
                )
                gq += 1
                br = sp.tile([128, D], mybir.dt.float32, tag="bred")
                nc.vector.tensor_tensor(
                    out=bg[:, : 8 * D], in0=bg[:, : 8 * D],
                    in1=bg[:, 8 * D : 16 * D],
                    op=mybir.AluOpType.max,
                )
                nc.vector.tensor_reduce(
                    out=br[:],
                    in_=bg[:, : 8 * D].rearrange("p (t d) -> p d t", d=D),
                    axis=mybir.AxisListType.X,
                    op=mybir.AluOpType.max,
                )
                nc.sync.dma_start(out=pb_out[m * 128 : (m + 1) * 128, :], in_=br[:])

            for m in range(NBATCH):
                # ---- main pass: sorted batch m, passes [(rpe, T), ...] ---
                it = ip.tile([128, CM], mybir.dt.int16, tag="midx")
                nc.sync.dma_start(out=it[:], in_=mi_in[m])
                last_batch = m == NBATCH - 1
                if last_batch:
                    boundary(m)
                acc = None
                coff = 0  # column offset into the index table
                ri = 0
                for rpe, T in tms[m]:
                    if T == 0:
                        continue
                    elem = rpe * D  # floats per gather element
                    thmax = 8192 // elem  # elements per chunk (32KB tile)
                    src = blocks8 if rpe == 8 else blocks32
                    chunks = [
                        (c0, min(thmax, T - c0)) for c0 in range(0, T, thmax)
                    ]
                    # split the first batch's first call into a small primer
                    # so the first reduce starts as early as possible
                    if m == 0 and acc is None and chunks[0][1] > 1:
                        chunks = [(0, 1), (1, chunks[0][1] - 1)] + chunks[1:]
                    for c0, TH in chunks:
                        g = gp.tile([128, 8192], mybir.dt.float32, tag="gath")
                        nc.gpsimd.dma_gather(
                            out_ap=g[:, : TH * elem].rearrange(
                                "p (t e) -> p t e", e=elem
                            ),
                            in_ap=src,
                            idxs_ap=it[:, coff + c0 * 8 : coff + (c0 + TH) * 8],
                            num_idxs=TH * 128,
                            num_idxs_reg=TH * 128,
                            elem_size=elem,
                            elem_step=1024,
                            # first calls stay on one queue: FIFO completion,
                            # so the pipeline ramps instead of all transfers
                            # sharing bandwidth and finishing together
                            queue_num=0 if gq < 5 else gq % 3,
                            single_packet=False,
                        )
                        gq += 1
                        r = sp.tile([128, D], mybir.dt.float32, tag=f"red{ri % 8}")
                        ri += 1
                        rows = TH * rpe
                        # 2-port TT halvings (1.05 ns/elem) instead of strided
                        # tensor_reduce (1.67 ns/elem)
                        while rows % 2 == 0 and rows > 8:
                            half = rows // 2 * D
                            nc.vector.tensor_tensor(
                                out=g[:, :half], in0=g[:, :half],
                                in1=g[:, half : 2 * half],
                                op=mybir.AluOpType.max,
                            )
                            rows //= 2
                        nc.vector.tensor_reduce(
                            out=r[:],
                            in_=g[:, : rows * D].rearrange(
                                "p (t d) -> p d t", d=D
                            ),
                            axis=mybir.AxisListType.X,
                            op=mybir.AluOpType.max,
                        )
                        if acc is None:
                            acc = r
                        else:
                            nc.vector.tensor_tensor(
                                out=acc[:], in0=acc[:], in1=r[:],
                                op=mybir.AluOpType.max,
                            )
                    coff += T * 8
                nc.sync.dma_start(out=pm_out[m * 128 : (m + 1) * 128, :], in_=acc[:])
                if not last_batch:
                    boundary(m)

    if not nc.is_finalized():
        nc.finalize()
    return nc


def kernel(features, segment_ids, num_clusters):
    global _last_results
    from concourse.bass_utils import run_bass_kernel_spmd

    features = np.ascontiguousarray(np.asarray(features, dtype=np.float32))
    ids = np.asarray(segment_ids).astype(np.int64)
    nclusters = int(num_clusters)
    assert features.shape == (N_POINTS, D), features.shape
    assert ids.shape == (N_POINTS,)
    assert nclusters == N_CLUSTERS

    # --- host cluster metadata -------------------------------------------
    gstart = np.searchsorted(ids, np.arange(nclusters), side="left")
    gend = np.searchsorted(ids, np.arange(nclusters) + 1, side="left")
    gcounts = gend - gstart

    core_meta = []  # per core dict
    for c in range(N_CORES):
        r0, r1 = c * RPC, (c + 1) * RPC
        cl_lo, cl_hi = int(ids[r0]), int(ids[r1 - 1])
        ncl = cl_hi - cl_lo + 1
        assert ncl <= NCL, f"core {c}: {ncl} local clusters > {NCL}"
        s = np.clip(gstart[cl_lo : cl_hi + 1], r0, r1) - r0
        e = np.clip(gend[cl_lo : cl_hi + 1], r0, r1) - r0
        s_pad = np.zeros(NCL, dtype=np.int64)
        e_pad = np.zeros(NCL, dtype=np.int64)
        s_pad[:ncl] = s
        e_pad[:ncl] = e
        cnt = e_pad - s_pad
        a0 = (s_pad + 7) // 8  # first aligned block fully inside
        a1 = e_pad // 8  # one past last aligned block fully inside
        nb = np.maximum(a1 - a0, 0)
        nb[cnt == 0] = 0
        order = np.argsort(nb, kind="stable")  # sorted batches for main pass
        core_meta.append(
            dict(cl_lo=cl_lo, ncl=ncl, s=s_pad, e=e_pad, cnt=cnt,
                 a0=a0, nb=nb, order=order)
        )

    # Main-pass per-batch passes shared across cores.  Batch 0 holds every
    # nb<4 cluster -> 8-row elements; batches 1+ are all nb>=4 -> 32-row
    # elements with T = max ceil(nb/4), the last element tail-anchored.
    tms = []
    for m in range(NBATCH):
        if m == 0:
            T = 1
            for cm in core_meta:
                sel = cm["order"][:128]
                T = max(T, int(cm["nb"][sel].max()))
            tms.append([(8, T)])
        else:
            T = 1
            for cm in core_meta:
                sel = cm["order"][m * 128 : (m + 1) * 128]
                nb = cm["nb"][sel]
                assert int(nb.min()) >= 4, (m, int(nb.min()))
                T = max(T, int(-(-nb.max() // 4)))
            tms.append([(32, T)])
    CM = max(sum(t for _, t in ps) for ps in tms) * 8

    # Boundary windows (consecutive batches, shared across cores).
    bwindows = []
    for m in range(NBATCH):
        jj = slice(m * 128, (m + 1) * 128)
        wmin = RPC
        for cm in core_meta:
            act = cm["cnt"][jj] >= 8
            if act.any():
                wmin = min(wmin, int(cm["s"][jj][act].min()))
        w = max(0, min(wmin, RPC - WINDOW))
        bwindows.append(w)

    def wrap_calls(V, T):
        """V[j] with j = t*128+p for t in [0,T) -> wrapped int16 [128, T*8]."""
        tab = V.reshape(T * 8, 16).T  # [16, T*8]; col m covers V[m*16:(m+1)*16]
        return np.tile(tab, (8, 1))

    # --- main-pass tables -------------------------------------------------
    midx_all, bidx_all = [], []
    for cm in core_meta:
        g = np.zeros((NBATCH, 128, CM), dtype=np.int16)
        for m in range(NBATCH):
            sel = cm["order"][m * 128 : (m + 1) * 128]
            a0 = cm["a0"][sel]
            nb = cm["nb"][sel]
            coff = 0
            for rpe, T in tms[m]:
                if T == 0:
                    continue
                t = np.arange(T)[None, :]
                if rpe == 32:
                    # anchor t: a0 + min(4t, nb-4); covers [a0, a1) exactly
                    # with the final element tail-anchored, pads repeat last
                    blk = a0[:, None] + np.minimum(4 * t, (nb - 4)[:, None])
                    assert blk.max() < NBLKS - 3
                else:
                    blk = a0[:, None] + t
                    last = np.where(nb > 0, a0 + nb - 1, 0)
                    blk = np.where(t >= nb[:, None], last[:, None], blk)
                    blk = np.where((nb[:, None] > 0), blk, 0)
                    assert blk.max() < NBLKS
                assert blk.min() >= 0
                V = blk.astype(np.int16).T.reshape(-1)  # j = t*128 + p
                g[m, :, coff : coff + T * 8] = wrap_calls(V, T)
                coff += T * 8
        midx_all.append(g)

        # boundary tables: consecutive batches, head+tail 8-row anchors
        b = np.zeros((NBATCH, 128, NB2 * 8), dtype=np.int16)
        for m in range(NBATCH):
            jj = slice(m * 128, (m + 1) * 128)
            s = cm["s"][jj]
            e = cm["e"][jj]
            cnt = cm["cnt"][jj]
            w = bwindows[m]
            ok = cnt >= 8
            head = np.where(ok, s - w, 0)
            tail = np.where(ok, e - 8 - w, 0)
            rel = np.stack([head, tail])  # [2, 128], j = t*128+p
            assert rel.min() >= 0 and rel.max() < WINDOW - 7, (m, rel.min(), rel.max())
            b[m] = wrap_calls(rel.astype(np.int16).reshape(-1), NB2)
        bidx_all.append(b)

    # --- build + run ------------------------------------------------------
    nc = _build_program(tms, bwindows)
    in_maps = [
        {
            "features": features[c * RPC : (c + 1) * RPC],
            "midx": midx_all[c],
            "bidx": bidx_all[c],
        }
        for c in range(N_CORES)
    ]
    res = run_bass_kernel_spmd(nc, in_maps, list(range(N_CORES)))
    _last_results = res

    # --- host combine -----------------------------------------------------
    full = np.full((nclusters, D), -np.inf, dtype=np.float32)
    for c in range(N_CORES):
        cm = core_meta[c]
        cl_lo, ncl = cm["cl_lo"], cm["ncl"]
        pm = res.results[c]["pmain"]  # [NCL, D] in sorted order
        pb = res.results[c]["pbnd"]  # [NCL, D] in consecutive order
        order = cm["order"]
        nb_sorted = cm["nb"][order]
        valid_m = (nb_sorted > 0) & (order < ncl)
        rows = cl_lo + order[valid_m]
        np.maximum.at(full, rows, pm[valid_m])
        valid_b = (cm["cnt"][:NCL] >= 8) & (np.arange(NCL) < ncl)
        rows = cl_lo + np.nonzero(valid_b)[0]
        np.maximum.at(full, rows, pb[valid_b])
        # tiny clusters (0 < cnt < 8, shard edges only): host-computed
        tiny = np.nonzero((cm["cnt"][:NCL] > 0) & (cm["cnt"][:NCL] < 8)
                          & (np.arange(NCL) < ncl))[0]
        r0 = c * RPC
        for j in tiny:
            rows_f = features[r0 + cm["s"][j] : r0 + cm["e"][j]]
            full[cl_lo + j] = np.maximum(full[cl_lo + j], rows_f.max(axis=0))
    full[gcounts == 0] = -np.inf
    return full
